# revision 16
# baseline (speedup 1.0000x reference)
"""MoE FFN (routed top-1, E=4) Trainium2 Bass kernel.

Strategy
--------
Data-parallel: 8192 tokens sharded as 1024 tokens per core; expert weights
replicated. Per core, everything runs on-device as dense matmuls (no dynamic
control flow, no indirect DMA):

 1. Router: logits = x @ router_w + router_b (fp32 matmul; argmax of softmax
    == argmax of logits). One-hot mask via reduce_max + is_equal.
 2. Rank of each token within its expert via a cumsum matmul
    (upper-triangular-ones constant), giving each token a destination slot
    dest[t] = expert*CAP + rank-1 with fixed per-expert capacity CAP=384
    (mean load is 256, CAP is ~9 sigma above it).
 3. Gather tokens into expert-contiguous, feature-major layout with a
    permutation matmul: x_perm[D, slots] = x_tm.T @ G^T, where
    G^T[t, j] = (j == dest[t]) is built with a per-partition iota compare.
 4. Per expert e: h = gelu(x_perm[:, e] @ w1[e] + b1[e]) (feature-major,
    bias fused into the activation instruction), y = h.T @ w2[e] + ...
    (token-major out).
 5. Un-permute + b2 in one accumulation group:
    out = G^T(transposed-role) @ y_perm + mask @ b2.

FFN matmuls run in bf16 with fp32 PSUM accumulation; the router runs fp32.
"""

import numpy as np
import ml_dtypes
from contextlib import ExitStack

import concourse.bass as bass
import concourse.tile as tile
from concourse import bacc, mybir
from concourse.bass import ts
from concourse.bass_utils import run_bass_kernel_spmd

# Problem dims (hardcoded per contract)
D, H, E = 1024, 4096, 4
B, S = 4, 2048
NCORES = 8
T = (B * S) // NCORES  # 1024 tokens per core
CAP = 384              # per-expert slot capacity
CT = E * CAP           # 1536 permuted slots
TK = T // 128          # 8 token tiles
DK = D // 128          # 8 dim tiles
HK = H // 128          # 32 hidden tiles
CTK = CT // 128        # 12 slot tiles
TM = CAP // 128        # 3 token m-tiles per expert group

BF = mybir.dt.bfloat16
F32 = mybir.dt.float32
bfnp = ml_dtypes.bfloat16

_GELU = mybir.ActivationFunctionType.Gelu
_EQ = mybir.AluOpType.is_equal

# Overridable for CoreSim (which lacks a Gelu implementation).
ACT_FUNC = _GELU


def build_bass():
    nc = bacc.Bacc(
        "TRN2",
        target_bir_lowering=False,
        debug=False,
        enable_asserts=True,
        num_devices=NCORES,
    )

    def din(name, shape, dt):
        return nc.dram_tensor(name, shape, dt, kind="ExternalInput").ap()

    x_tm = din("x_tm", [T, D], BF)           # token-major x (bf16)
    xT = din("xT", [D, T], F32)              # feature-major x (fp32, router)
    rw = din("rw", [D, E], F32)
    rb_rep = din("rb_rep", [128, E], F32)    # router_b replicated over partitions
    w1 = din("w1", [E, D, H], BF)
    b1t = din("b1t", [E, 128, HK], F32)      # b1[e] as [128, HK] (partition-major)
    w2 = din("w2", [E, H, D], BF)
    b2 = din("b2", [E, D], BF)
    utri = din("utri", [128, 128], BF)       # upper-triangular ones (incl diag)
    onesq = din("onesq", [128, 128], BF)     # all-ones square
    ident = din("ident", [128, 128], BF)     # identity (PE transpose)
    iota_rep = din("iota_rep", [128, CT], F32)  # rows = 0..CT-1
    offs_rep = din("offs_rep", [128, E], F32)   # rows = e*CAP - 1
    iota_hi = din("iota_hi", [T, 1], BF)     # (t//4)*4 - 1024  (bf16-exact)
    iota_lo = din("iota_lo", [T, 1], BF)     # t%4

    out = nc.dram_tensor("out", [T, D], F32, kind="ExternalOutput").ap()
    pv_scratch = nc.dram_tensor("pv_scratch", [1, CT], F32).ap()

    x_tm_r = x_tm.rearrange("(t p) d -> t p d", p=128)
    xT_r = xT.rearrange("(k p) t -> k p t", p=128)
    rw_r = rw.rearrange("(k p) e -> p k e", p=128)
    out_r = out.rearrange("(t p) d -> t p d", p=128)

    with tile.TileContext(nc) as tc, ExitStack() as ctx:
        pool = lambda name, bufs: ctx.enter_context(tc.tile_pool(name=name, bufs=bufs))
        ppool = lambda name, bufs: ctx.enter_context(
            tc.tile_pool(name=name, bufs=bufs, space="PSUM")
        )

        consts = pool("consts", 1)
        utri_t = consts.tile([128, 128], BF, tag="utri")
        nc.sync.dma_start(utri_t[:], utri)
        ones_t = consts.tile([128, 128], BF, tag="ones")
        nc.sync.dma_start(ones_t[:], onesq)
        ident_t = consts.tile([128, 128], BF, tag="ident")
        nc.sync.dma_start(ident_t[:], ident)
        iota_t = consts.tile([128, CT], F32, tag="iota")
        nc.sync.dma_start(iota_t[:], iota_rep)
        offs_t = consts.tile([128, E], F32, tag="offs")
        nc.sync.dma_start(offs_t[:], offs_rep)
        rb_t = consts.tile([128, E], F32, tag="rb")
        nc.sync.dma_start(rb_t[:], rb_rep)
        rw_t = consts.tile([128, DK * E], F32, tag="rw")
        nc.sync.dma_start(rw_t[:].rearrange("p (k e) -> p k e", k=DK), rw_r)
        b2_t = consts.tile([E, D], BF, tag="b2")
        nc.sync.dma_start(b2_t[:], b2)
        b1_t = consts.tile([128, E * HK], F32, tag="b1")
        nc.sync.dma_start(b1_t[:].rearrange("p (e m) -> p e m", e=E), b1t.rearrange("e p m -> p e m"))
        ihi_t = consts.tile([128, TK], BF, tag="ihi")
        nc.sync.dma_start(ihi_t[:], iota_hi.rearrange("(k p) o -> p (k o)", p=128))
        ilo_t = consts.tile([128, TK], BF, tag="ilo")
        nc.sync.dma_start(ilo_t[:], iota_lo.rearrange("(k p) o -> p (k o)", p=128))

        # ---- persistent big activations ----
        big = pool("big", 1)
        xtm_t = big.tile([128, TK * D], BF, tag="xtm")  # [p, (tk, d)]
        for tk in range(TK):
            nc.sync.dma_start(xtm_t[:, ts(tk, D)], x_tm_r[tk])
        gt_t = big.tile([128, TK * CT], BF, tag="gt")    # G^T tiles [p=tok, (tk, slot)]
        xperm_t = big.tile([128, DK * CT], BF, tag="xperm")  # [p=dim, (dk, slot)]
        y_t = big.tile([128, CTK * D], BF, tag="y")      # [p=slot, (ct, d)]
        maskT_t = big.tile([4, T], BF, tag="maskT")

        small = pool("small", 1)
        mask_bf = [small.tile([128, E], BF, tag=f"mask{i}", name=f"mask{i}") for i in range(TK)]
        mask_f32 = [small.tile([128, E], F32, tag=f"maskf{i}", name=f"maskf{i}") for i in range(TK)]
        dest_t = [small.tile([128, 1], F32, tag=f"dest{i}", name=f"dest{i}") for i in range(TK)]
        pv_sb = small.tile([1, CT], F32, tag="pv")
        pvcol = [small.tile([128, 1], F32, tag=f"pvc{i}", name=f"pvc{i}") for i in range(CTK)]

        # ================= Phase A: router + dest =================
        with tc.tile_pool(name="xT", bufs=1) as xT_pool, \
             tc.tile_pool(name="psA", bufs=4, space="PSUM") as psA, \
             tc.tile_pool(name="sbA", bufs=4) as sbA:
            xT_tiles = []
            for dk in range(DK):
                t = xT_pool.tile([128, T], F32, tag=f"xT{dk}")
                nc.sync.dma_start(t[:], xT_r[dk])
                xT_tiles.append(t)

            logits = [sbA.tile([128, E], F32, tag=f"lg{tm}", name=f"lg{tm}") for tm in range(TK)]
            for tm in range(TK):
                ps = psA.tile([128, E], F32, tag="ps_l")
                for dk in range(DK):
                    nc.tensor.matmul(
                        ps[:],
                        xT_tiles[dk][:, ts(tm, 128)],
                        rw_t[:, ts(dk, E)],
                        start=(dk == 0),
                        stop=(dk == DK - 1),
                    )
                nc.vector.tensor_add(logits[tm][:], ps[:], rb_t[:])
                rmax = sbA.tile([128, 1], F32, tag="rmax")
                nc.vector.reduce_max(rmax[:], logits[tm][:], axis=mybir.AxisListType.X)
                nc.vector.tensor_scalar(mask_bf[tm][:], logits[tm][:], rmax[:], None, op0=_EQ)
                nc.vector.tensor_scalar(mask_f32[tm][:], logits[tm][:], rmax[:], None, op0=_EQ)

            # cumsum over tokens: cum = U^T @ mask
            for tm in range(TK):
                ps = psA.tile([128, E], F32, tag="ps_c")
                for tk in range(tm + 1):
                    nc.tensor.matmul(
                        ps[:],
                        (utri_t if tk == tm else ones_t)[:],
                        mask_bf[tk][:],
                        start=(tk == 0),
                        stop=(tk == tm),
                    )
                tmp = sbA.tile([128, E], F32, tag="tmpA")
                nc.vector.tensor_add(tmp[:], ps[:], offs_t[:])
                nc.vector.tensor_mul(tmp[:], tmp[:], mask_f32[tm][:])
                nc.vector.reduce_sum(dest_t[tm][:], tmp[:], axis=mybir.AxisListType.X)

        # ================= Phase B: G^T, perm_vec, gather =================
        for tk in range(TK):
            nc.vector.tensor_scalar(
                gt_t[:, ts(tk, CT)], iota_t[:], dest_t[tk][:], None, op0=_EQ
            )

        with tc.tile_pool(name="psB", bufs=4, space="PSUM") as psB:
            # perm_vec[j] = token index landing in slot j (sum of hi+lo parts)
            for sc in range(CT // 512):
                ps = psB.tile([1, 512], F32, tag="ps_pv")
                n = 0
                for part in (ihi_t, ilo_t):
                    for tk in range(TK):
                        nc.tensor.matmul(
                            ps[:],
                            part[:, tk : tk + 1],
                            gt_t[:, tk * CT + sc * 512 : tk * CT + (sc + 1) * 512],
                            start=(n == 0),
                            stop=(n == 2 * TK - 1),
                        )
                        n += 1
                # +1024 undoes the iota shift; empty slots land at 1024,
                # which matches no token in the G compare (out of range).
                nc.vector.tensor_scalar_add(pv_sb[:, ts(sc, 512)], ps[:], 1024.0)
                nc.sync.dma_start(pv_scratch[:, ts(sc, 512)], pv_sb[:, ts(sc, 512)])
            pv_r = pv_scratch.rearrange("o (c p) -> c p o", p=128)
            for ct in range(CTK):
                nc.sync.dma_start(pvcol[ct][:], pv_r[ct])

            # gather: x_perm[dk] = x_tm.T @ G^T
            for dm in range(DK):
                for sc in range(CT // 512):
                    ps = psB.tile([128, 512], F32, tag="ps_g")
                    for tk in range(TK):
                        nc.tensor.matmul(
                            ps[:],
                            xtm_t[:, tk * D + dm * 128 : tk * D + dm * 128 + 128],
                            gt_t[:, tk * CT + sc * 512 : tk * CT + (sc + 1) * 512],
                            start=(tk == 0),
                            stop=(tk == TK - 1),
                        )
                    nc.vector.tensor_copy(xperm_t[:, dm * CT + sc * 512 : dm * CT + (sc + 1) * 512], ps[:])

        # ================= Phase C: expert FFN =================
        with tc.tile_pool(name="w1p", bufs=4) as w1p, \
             tc.tile_pool(name="w2p", bufs=4) as w2p, \
             tc.tile_pool(name="hp", bufs=2) as hp, \
             tc.tile_pool(name="psh", bufs=2, space="PSUM") as psh, \
             tc.tile_pool(name="psy", bufs=1, space="PSUM") as psy:
            for e in range(E):
                h_sb = hp.tile([128, HK * CAP], BF, tag="h")
                for hm in range(HK):
                    w1c = w1p.tile([128, DK * 128], BF, tag="w1c")
                    nc.sync.dma_start(
                        w1c[:].rearrange("p (k h) -> p k h", k=DK),
                        w1[e].rearrange("(k p) h -> p k h", p=128)[:, :, ts(hm, 128)],
                    )
                    ps = psh.tile([128, CAP], F32, tag="ps_h")
                    for dk in range(DK):
                        nc.tensor.matmul(
                            ps[:],
                            w1c[:, ts(dk, 128)],
                            xperm_t[:, dk * CT + e * CAP : dk * CT + (e + 1) * CAP],
                            start=(dk == 0),
                            stop=(dk == DK - 1),
                        )
                    nc.scalar.activation(
                        h_sb[:, ts(hm, CAP)], ps[:], ACT_FUNC,
                        bias=b1_t[:, e * HK + hm : e * HK + hm + 1], scale=1.0,
                    )
                psy_t = [psy.tile([128, D], F32, tag=f"ps_y{tm}", name=f"ps_y{tm}") for tm in range(TM)]
                for kk in range(HK):
                    w2r = w2p.tile([128, D], BF, tag="w2r")
                    nc.sync.dma_start(w2r[:], w2[e, ts(kk, 128), :])
                    for tm in range(TM):
                        for nn in range(D // 512):
                            nc.tensor.matmul(
                                psy_t[tm][:, ts(nn, 512)],
                                h_sb[:, kk * CAP + tm * 128 : kk * CAP + tm * 128 + 128],
                                w2r[:, ts(nn, 512)],
                                start=(kk == 0),
                                stop=(kk == HK - 1),
                            )
                for tm in range(TM):
                    nc.vector.tensor_copy(y_t[:, ts(e * TM + tm, D)], psy_t[tm][:])

        # ================= Phase D: unpermute + b2 =================
        with tc.tile_pool(name="gp", bufs=1) as gp, \
             tc.tile_pool(name="psD", bufs=4, space="PSUM") as psD, \
             tc.tile_pool(name="outp", bufs=3) as outp:
            g_t = gp.tile([128, CTK * T], BF, tag="g")
            for ct in range(CTK):
                nc.vector.tensor_scalar(
                    g_t[:, ts(ct, T)], iota_t[:, :T], pvcol[ct][:], None, op0=_EQ
                )
            for tm in range(TK):
                psm = psD.tile([4, 128], BF, tag="ps_mt")
                nc.tensor.transpose(psm[:], mask_bf[tm][:], ident_t[:])
                nc.vector.tensor_copy(maskT_t[:, ts(tm, 128)], psm[:])
            for tm in range(TK):
                o_sb = outp.tile([128, D], F32, tag="o")
                for nn in range(D // 512):
                    ps = psD.tile([128, 512], F32, tag="ps_o")
                    for ct in range(CTK):
                        nc.tensor.matmul(
                            ps[:],
                            g_t[:, ct * T + tm * 128 : ct * T + tm * 128 + 128],
                            y_t[:, ct * D + nn * 512 : ct * D + (nn + 1) * 512],
                            start=(ct == 0),
                            stop=False,
                        )
                    nc.tensor.matmul(
                        ps[:],
                        maskT_t[:, ts(tm, 128)],
                        b2_t[:, ts(nn, 512)],
                        start=False,
                        stop=True,
                    )
                    nc.vector.tensor_copy(o_sb[:, ts(nn, 512)], ps[:])
                nc.sync.dma_start(out_r[tm], o_sb[:])

    nc.compile()
    return nc


def make_in_maps(inputs):
    x = np.asarray(inputs["x"], np.float32).reshape(B * S, D)
    rw = np.asarray(inputs["router_w"], np.float32)
    rb = np.asarray(inputs["router_b"], np.float32)
    w1 = np.asarray(inputs["w1"], np.float32)
    b1 = np.asarray(inputs["b1"], np.float32)
    w2 = np.asarray(inputs["w2"], np.float32)
    b2 = np.asarray(inputs["b2"], np.float32)

    w1b = np.ascontiguousarray(w1.astype(bfnp))
    w2b = np.ascontiguousarray(w2.astype(bfnp))
    b2b = np.ascontiguousarray(b2.astype(bfnp))
    b1t = np.ascontiguousarray(b1.reshape(E, HK, 128).transpose(0, 2, 1)).astype(np.float32)
    rb_rep = np.tile(rb[None, :], (128, 1)).astype(np.float32)
    utri_m = np.triu(np.ones((128, 128))).astype(bfnp)
    ones_m = np.ones((128, 128), dtype=bfnp)
    ident_m = np.eye(128).astype(bfnp)
    iota_rep = np.tile(np.arange(CT, dtype=np.float32)[None, :], (128, 1))
    offs_rep = np.tile(
        (np.arange(E, dtype=np.float32) * CAP - 1.0)[None, :], (128, 1)
    ).astype(np.float32)
    tt = np.arange(T)
    iota_hi = ((tt // 4) * 4 - 1024).astype(bfnp).reshape(T, 1)
    iota_lo = (tt % 4).astype(bfnp).reshape(T, 1)

    in_maps = []
    for c in range(NCORES):
        xs = x[c * T : (c + 1) * T]
        in_maps.append(
            {
                "x_tm": np.ascontiguousarray(xs.astype(bfnp)),
                "xT": np.ascontiguousarray(xs.T),
                "rw": rw,
                "rb_rep": rb_rep,
                "w1": w1b,
                "b1t": b1t,
                "w2": w2b,
                "b2": b2b,
                "utri": utri_m,
                "onesq": ones_m,
                "ident": ident_m,
                "iota_rep": iota_rep,
                "offs_rep": offs_rep,
                "iota_hi": iota_hi,
                "iota_lo": iota_lo,
            }
        )
    return in_maps


_NC_CACHE = None


def get_nc():
    global _NC_CACHE
    if _NC_CACHE is None:
        _NC_CACHE = build_bass()
    return _NC_CACHE


def kernel(**inputs):
    nc = get_nc()
    in_maps = make_in_maps(inputs)
    res = run_bass_kernel_spmd(nc, in_maps, list(range(NCORES)))
    outs = [np.asarray(res.results[c]["out"], np.float32) for c in range(NCORES)]
    return np.concatenate(outs, axis=0).reshape(B, S, D)
